# revision 10
# baseline (speedup 1.0000x reference)
"""MultiHeadSelfAttentionWithRoPE on 8 TRN2 NeuronCores.

v3 hybrid sharding: the collective subsystem has a fixed ~80us startup
latency from exec start plus ~25-30us per AllGather, so gathering all
K/V is collective-bound.  Instead each core REPLICATES the K/V
projection for kv-block groups 0,1 (blocks b with b mod 4 < 2; 16
blocks, packed host-side into xTr in slot order so attention starts at
~30us) and shards + AllGathers only groups 2,3 (own sub-blocks 2,3;
two ~0.8MB collectives whose completion is hidden behind the local
groups' attention).

Queries stay round-robin sharded (core c owns rows {c, c+8, ...}) so
causal work is balanced and the program is SPMD-identical; per-core
differences enter only through input data.

Layouts:
  QT  [128, eo(2), quad(3), 512] bf16 — head-dim permutation folded into
      wq/wk column order on host (evens/odds split, 4 heads per 32-row
      group); scores are invariant to a consistent q/k permutation.
  KT2 [128, slot(32), eo(2), quad(3), 128] bf16 — slot 8j+r holds kv
      block b = 4r+j.  Slots 0:16 written locally (repl proj), slots
      16:32 by the AG readback (one contiguous-run DMA per tensor).
  VSB2 [128, slot(32), head(12), 65] bf16 — col 64 is ones (memset for
      local slots, staged through the AG for gathered slots) so the
      softmax denominator falls out of the PV matmul.
  ST  [128, 4, 512] f32 PSUM — one tile per kv block; 4 head matmuls
      per eo wave run concurrently via tile_position=(32a,0); one Exp
      call covers all 4 heads (attention is ACT/exp-bound).
  OT  [65, 4, 512] f32 PSUM per (group, quad) accumulated over the
      group's 8 blocks, drained (add) into bf16 SBUF accumulators
      OTacc[g]; normalize via the [128,16] spread-reciprocal +
      DRAM-broadcast bounce; out = OT.T @ woT.
"""

import numpy as np

D = 768
S = 4096
H = 12
HD = 64
HD2 = 32
NCORES = 8
QPC = S // NCORES          # 512 query rows per core
NKV = S // 128             # 32 kv blocks
NDS = D // 128             # 6 d-slices
SCALE = float(1.0 / np.sqrt(HD))

_CACHE = {}
last_exec_time_ns = None
last_results = None


def _head_perm():
    """Column permutation for wq/wk: slab s = 3*eo + quad, partition p =
    32*a + i  ->  original dim e = 64*(4*quad + a) + 2*i + eo."""
    perm = np.zeros(D, dtype=np.int64)
    for s in range(6):
        eo, quad = divmod(s, 3)
        for p in range(128):
            a, i = divmod(p, 32)
            perm[128 * s + p] = 64 * (4 * quad + a) + 2 * i + eo
    return perm


def _repl_cols():
    """xTr column order: slot-major.  Slot t (t=0..15) holds kv block
    b = 4*(t%8) + t//8, so repl chunk ci (512 cols) covers the four
    adjacent slots 4ci..4ci+3."""
    cols = []
    for t in range(16):
        b = 4 * (t % 8) + t // 8
        cols.extend(range(128 * b, 128 * (b + 1)))
    return np.array(cols, dtype=np.int64)


def _build_program():
    import concourse.mybir as mybir
    import concourse.tile as tile
    from concourse import bacc
    from contextlib import ExitStack

    dt = mybir.dt
    bf = dt.bfloat16
    f32 = dt.float32
    nc = bacc.Bacc("TRN2", target_bir_lowering=False, debug=False,
                   num_devices=NCORES)

    def din(name, shape, dtype):
        return nc.dram_tensor(name, shape, dtype, kind="ExternalInput").ap()

    xkv2_d = din("xkv2", [D, 256], bf)     # own chunk cols 256:512
    xTr_d = din("xTr", [128, NDS, 2048], bf)   # repl cols, ds-split rows
    xq_d = din("xq", [D, QPC], bf)         # strided q slice, transposed
    wqT_d = din("wqT", [D, D], bf)         # permuted cols
    wkT_d = din("wkT", [D, D], bf)         # permuted cols
    wvT_d = din("wvT", [D, D], bf)         # natural
    woT_d = din("woT", [D, D], bf)         # wo.T natural
    cosK2_d = din("cosK2", [128, 256], bf)     # own sub-blocks 2,3
    sinK2_d = din("sinK2", [128, 256], bf)
    cosKr_d = din("cosKr", [128, 2048], bf)    # repl cols (slot order)
    sinKr_d = din("sinKr", [128, 2048], bf)
    cosQ_d = din("cosQ", [128, QPC], bf)
    sinQ_d = din("sinQ", [128, QPC], bf)
    mask_d = din("mask", [128, 64], bf)
    out_d = nc.dram_tensor("out", [QPC, D], bf, kind="ExternalOutput").ap()

    RG = [list(range(NCORES))]

    with tile.TileContext(nc) as tc, ExitStack() as ctx:
        # ---- long-lived SBUF ----
        P_LL = ctx.enter_context(tc.tile_pool(name="ll", bufs=1))
        KT2 = P_LL.tile([128, NKV, 2, 3, 128], bf)           # 6.3 MB
        QT = P_LL.tile([128, 2, 3, QPC], bf)                 # 0.8 MB
        VSB2 = P_LL.tile([128, NKV, H, HD + 1], bf)          # 6.4 MB
        OTSBq = [P_LL.tile([128, 2, QPC], bf, name=f"otsb{gq}")
                 for gq in range(3)]
        OTacc = [P_LL.tile([65, 4, QPC], bf, name=f"otacc{gq}")
                 for gq in range(3)]
        Ktmp = P_LL.tile([128, 2, 3, 256], bf)   # own roped K (sb 2,3)
        msk = P_LL.tile([128, 4, 16], bf)
        ones12 = P_LL.tile([128, H], bf)
        nc.sync.dma_start(msk[:], mask_d.rearrange("p (s f) -> p s f", s=4))
        nc.gpsimd.memset(ones12[:], 1.0)
        # ones column for the locally-written V slots
        nc.gpsimd.memset(VSB2[:, 0:16, :, HD:HD + 1], 1.0)

        # ---- DRAM staging for the two AllGathers (sub-blocks 2,3) ----
        P_DR = ctx.enter_context(tc.tile_pool(name="dr", bufs=1,
                                              space="DRAM"))
        KCOLS = 768
        VCOLS = H * (HD + 1)
        KVC = KCOLS + VCOLS
        kin = {j: P_DR.tile([128, KVC], bf, name=f"kin{j}", tag=f"kin{j}")
               for j in (2, 3)}
        kout = {j: P_DR.tile([128 * NCORES, KVC], bf, name=f"kout{j}",
                             tag=f"kout{j}", addr_space="Shared")
                for j in (2, 3)}

        # ---- transient SBUF pools ----
        P_W = ctx.enter_context(tc.tile_pool(name="wt", bufs=2))
        P_X = ctx.enter_context(tc.tile_pool(name="xs", bufs=1))
        P_RT = ctx.enter_context(tc.tile_pool(name="rt", bufs=1))
        P_CS = ctx.enter_context(tc.tile_pool(name="cs", bufs=1))
        P_V = ctx.enter_context(tc.tile_pool(name="vst", bufs=2))
        P_P = ctx.enter_context(tc.tile_pool(name="pp", bufs=2))
        P_N = ctx.enter_context(tc.tile_pool(name="nrm", bufs=1))
        P_O = ctx.enter_context(tc.tile_pool(name="outs", bufs=1))

        def load_w(dram):
            # scalar-engine DMA queue, parallel to the sync-engine queue
            w = P_W.tile([128, NDS, D], bf, tag="w")
            for ds in range(NDS):
                nc.scalar.dma_start(w[:, ds, :],
                                    dram[128 * ds:128 * (ds + 1), :])
            return w

        def rope2(de, do, src_e_ps, src_o_ps, cos_t, sin_t, n, rs=None):
            """de/do bf16 (n elems/part) <- rotate psum pair by (cos,sin).
            If rs is given, de/do are [128, rs, n//rs]-shaped APs and the
            flat temporaries are reshaped to match."""
            def shp(ap):
                if rs is None:
                    return ap
                return ap.rearrange("p (s c) -> p s c", s=rs)
            se = P_RT.tile([128, n], bf, tag="se", bufs=2)
            so = P_RT.tile([128, n], bf, tag="so", bufs=2)
            nc.vector.tensor_copy(se[:], src_e_ps)
            nc.vector.tensor_copy(so[:], src_o_ps)
            t1 = P_RT.tile([128, n], bf, tag="t1", bufs=2)
            t2 = P_RT.tile([128, n], bf, tag="t2", bufs=2)
            nc.vector.tensor_mul(t1[:], se[:], cos_t)
            nc.vector.tensor_mul(t2[:], so[:], sin_t)
            nc.vector.tensor_sub(de, shp(t1[:]), shp(t2[:]))
            t3 = P_RT.tile([128, n], bf, tag="t1", bufs=2)
            t4 = P_RT.tile([128, n], bf, tag="t2", bufs=2)
            nc.vector.tensor_mul(t3[:], se[:], sin_t)
            nc.vector.tensor_mul(t4[:], so[:], cos_t)
            nc.vector.tensor_add(do, shp(t3[:]), shp(t4[:]))

        # ============ phase A: projections ============================
        with tc.tile_pool(name="pps", bufs=2, space="PSUM") as P_PS:
            # Q-proj inputs first: xq on the sync queue, wq at the
            # head of the scalar queue, so attention's gating input QT
            # is ready early
            xqs = []
            for ds in range(NDS):
                xq = P_X.tile([128, QPC], bf, tag="xq", bufs=6,
                              name=f"xq{ds}")
                nc.sync.dma_start(xq[:], xq_d[128 * ds:128 * (ds + 1), :])
                xqs.append(xq)
            cq = P_CS.tile([128, QPC], bf, tag="cq")
            sq = P_CS.tile([128, QPC], bf, tag="sq")
            nc.sync.dma_start(cq[:], cosQ_d)
            nc.sync.dma_start(sq[:], sinQ_d)
            wq_sb = load_w(wqT_d)
            wk_sb = load_w(wkT_d)
            wv_sb = load_w(wvT_d)
            xkv2 = []
            for ds in range(NDS):
                xt = P_X.tile([128, 256], bf, tag="xk2", bufs=6,
                              name=f"xkv2_{ds}")
                nc.sync.dma_start(xt[:], xkv2_d[128 * ds:128 * (ds + 1), :])
                xkv2.append(xt)
            ck2 = P_CS.tile([128, 256], bf, tag="ck2")
            sk2 = P_CS.tile([128, 256], bf, tag="sk2")
            nc.sync.dma_start(ck2[:], cosK2_d)
            nc.sync.dma_start(sk2[:], sinK2_d)

            # ---- Q projection ----
            for quad in range(3):
                pe = P_PS.tile([128, QPC], f32, tag="kpsE")
                po = P_PS.tile([128, QPC], f32, tag="kpsO")
                for s, ps in ((quad, pe), (3 + quad, po)):
                    for ds in range(NDS):
                        nc.tensor.matmul(
                            ps[:], wq_sb[:, ds, 128 * s:128 * (s + 1)],
                            xqs[ds][:], start=(ds == 0),
                            stop=(ds == NDS - 1))
                rope2(QT[:, 0, quad, :], QT[:, 1, quad, :],
                      pe[:], po[:], cq[:], sq[:], QPC)

            # ---- own K proj (cols 256:512 of own chunk) + rope ----
            for quad in range(3):
                pe = P_PS.tile([128, 256], f32, tag="kpsE")
                po = P_PS.tile([128, 256], f32, tag="kpsO")
                for s, ps in ((quad, pe), (3 + quad, po)):
                    for ds in range(NDS):
                        nc.tensor.matmul(
                            ps[:], wk_sb[:, ds, 128 * s:128 * (s + 1)],
                            xkv2[ds][:], start=(ds == 0),
                            stop=(ds == NDS - 1))
                rope2(Ktmp[:, 0, quad, :], Ktmp[:, 1, quad, :],
                      pe[:], po[:], ck2[:], sk2[:], 256)
            for sb in (2, 3):
                lo = 128 * (sb - 2)
                nc.sync.dma_start(
                    kin[sb][:, 0:KCOLS].rearrange(
                        "p (a b c) -> p a b c", a=2, b=3),
                    Ktmp[:, :, :, lo:lo + 128])
                vst = P_V.tile([128, H, HD + 1], bf, tag="vst")
                nc.vector.tensor_copy(
                    vst[:, :, HD:HD + 1],
                    ones12[:].rearrange("p (h o) -> p h o", o=1))
                for nh in range(2):
                    vps = P_PS.tile([128, 384], f32, tag="vps")
                    for ds in range(NDS):
                        nc.tensor.matmul(
                            vps[:], xkv2[ds][:, lo:lo + 128],
                            wv_sb[:, ds, 384 * nh:384 * (nh + 1)],
                            start=(ds == 0), stop=(ds == NDS - 1))
                    nc.vector.tensor_copy(
                        vst[:, 6 * nh:6 * (nh + 1), 0:HD],
                        vps[:].rearrange("p (h d) -> p h d", h=6))
                nc.sync.dma_start(
                    kin[sb][:, KCOLS:KVC],
                    vst[:].rearrange("p h d -> p (h d)"))
                nc.gpsimd.collective_compute(
                    "AllGather", mybir.AluOpType.bypass,
                    replica_groups=RG, ins=[kin[sb][:]],
                    outs=[kout[sb][:]])

            # ---- replicated proj for kv-block groups 0,1 ----
            for ci in range(4):                 # 512-col chunks
                c0 = 512 * ci
                s0 = 4 * ci                     # four adjacent slots
                xtr = P_X.tile([128, NDS, 512], bf, tag="xtr", bufs=3,
                               name=f"xtr{ci}")
                nc.sync.dma_start(xtr[:], xTr_d[:, :, c0:c0 + 512])
                ctr = P_CS.tile([128, 512], bf, tag="ctr", bufs=3)
                st_r = P_CS.tile([128, 512], bf, tag="str", bufs=3)
                nc.sync.dma_start(ctr[:], cosKr_d[:, c0:c0 + 512])
                nc.sync.dma_start(st_r[:], sinKr_d[:, c0:c0 + 512])
                for quad in range(3):
                    pe = P_PS.tile([128, 512], f32, tag="kpsE")
                    po = P_PS.tile([128, 512], f32, tag="kpsO")
                    for s, ps in ((quad, pe), (3 + quad, po)):
                        for ds in range(NDS):
                            nc.tensor.matmul(
                                ps[:], wk_sb[:, ds, 128 * s:128 * (s + 1)],
                                xtr[:, ds, :], start=(ds == 0),
                                stop=(ds == NDS - 1))
                    rope2(KT2[:, s0:s0 + 4, 0, quad, :],
                          KT2[:, s0:s0 + 4, 1, quad, :],
                          pe[:], po[:], ctr[:], st_r[:], 512, rs=4)
                for u in range(4):              # V proj per slot
                    slot = s0 + u
                    for nh in range(2):
                        vps = P_PS.tile([128, 384], f32, tag="vps")
                        for ds in range(NDS):
                            nc.tensor.matmul(
                                vps[:],
                                xtr[:, ds, 128 * u:128 * (u + 1)],
                                wv_sb[:, ds, 384 * nh:384 * (nh + 1)],
                                start=(ds == 0), stop=(ds == NDS - 1))
                        nc.vector.tensor_copy(
                            VSB2[:, slot, 6 * nh:6 * (nh + 1), 0:HD],
                            vps[:].rearrange("p (h d) -> p h d", h=6))

        # wo load overlaps attention (reuses wv's pool slot)
        wo_sb = load_w(woT_d)
        nrm_d = nc.dram_tensor("nrm_scratch", [3, 4 * QPC], bf,
                               kind="Internal").ap()

        # ============ phase C: attention ===============================
        rb4_g2 = None
        with tc.tile_pool(name="st", bufs=1, space="PSUM") as P_ST, \
             tc.tile_pool(name="ot", bufs=1, space="PSUM") as P_OT:
            for j in range(4):                 # block groups
                if j >= 2:
                    # readback of this group's gathered K/V
                    nc.sync.dma_start(
                        KT2[:, 8 * j:8 * (j + 1), :, :, :],
                        kout[j][:, 0:KCOLS].rearrange(
                            "(r p) (a b c) -> p r a b c",
                            r=NCORES, a=2, b=3))
                    nc.sync.dma_start(
                        VSB2[:, 8 * j:8 * (j + 1), :, :],
                        kout[j][:, KCOLS:KVC].rearrange(
                            "(r p) (h d) -> p r h d", r=NCORES, h=H))
                for g in range(3):             # head quads
                    otb = P_OT.tile([65, 4, QPC], f32, tag="ot")
                    prev = None

                    def pv_flush(g=g, otb=otb):
                        nonlocal prev
                        if prev is None:
                            return
                        pslot, pq0, pr, pp = prev
                        for a in range(4):
                            nc.tensor.matmul(
                                otb[:, a, pq0:QPC],
                                VSB2[:, pslot, 4 * g + a, :],
                                pp[:, a, 0:QPC - pq0], start=(pr == 0),
                                stop=(pr == 7))
                        prev = None

                    for r in range(NCORES):    # kv blocks of this group
                        b = 4 * r + j
                        slot = 8 * j + r
                        q0 = 16 * b
                        n = QPC - q0
                        st = P_ST.tile([128, 4, QPC], f32, tag="st")
                        for eo in range(2):
                            for a in range(4):
                                nc.tensor.matmul(
                                    st[:, a, 0:n],
                                    KT2[32 * a:32 * (a + 1), slot, eo, g, :],
                                    QT[32 * a:32 * (a + 1), eo, g, q0:QPC],
                                    start=(eo == 0), stop=(eo == 1),
                                    tile_position=(32 * a, 0))
                        pv_flush()
                        p = P_P.tile([128, 4, QPC], bf, tag="p")
                        nc.scalar.activation(
                            p[:, :, 0:n], st[:, :, 0:n],
                            mybir.ActivationFunctionType.Exp, scale=SCALE)
                        nc.vector.tensor_mul(
                            p[:, :, 0:16], p[:, :, 0:16], msk[:])
                        prev = (slot, q0, r, p)
                    pv_flush()

                    # drain this group's partial OT into the accumulator
                    qd = 16 * j
                    if j == 0:
                        nc.vector.tensor_copy(OTacc[g][:], otb[:])
                    else:
                        nc.vector.tensor_add(
                            OTacc[g][:, :, qd:], OTacc[g][:, :, qd:],
                            otb[:, :, qd:])

            # ---- normalize: spread-reciprocal of the denominator ----
            for g in range(3):
                d16 = P_N.tile([128, 16], bf, tag="d16")
                nc.sync.dma_start(
                    d16[:], OTacc[g][64:65, :, :].rearrange(
                        "p a n -> p (a n)"))
                r16 = P_N.tile([128, 16], f32, tag="r16")
                nc.vector.reciprocal(r16[:], d16[:])
                r16b = P_N.tile([128, 16], bf, tag="r16b")
                nc.vector.tensor_copy(r16b[:], r16[:])
                r1 = P_N.tile([1, 4 * QPC], bf, tag="r1")
                nc.sync.dma_start(r1[:], r16b[:])
                nc.sync.dma_start(nrm_d[g:g + 1, :], r1[:])
                rb = P_N.tile([128, 4 * QPC], bf, tag="rb")
                nc.sync.dma_start(
                    rb[:], nrm_d[g:g + 1, :].to_broadcast((128, 4 * QPC)))
                for half in range(2):
                    nc.vector.tensor_copy(
                        OTSBq[g][64 * half:64 * half + 64, :, :],
                        OTacc[g][0:64, half::2, :])
                rb4 = rb[:].rearrange("p (a q) -> p a q", a=4)
                if g < 2:
                    for a in range(4):
                        pb = 64 * (a % 2)
                        dst = OTSBq[g][pb:pb + 64, a // 2, :]
                        nc.vector.tensor_mul(dst, dst,
                                             rb4[pb:pb + 64, a, :])
                else:
                    # defer the last quad's normalize into phase D so it
                    # pipelines with the output projection
                    rb4_g2 = rb4

        # ============ phase D: output projection =======================
        with tc.tile_pool(name="pd", bufs=4, space="PSUM") as P_PD:
            for jj in range(4):                 # q sub-tiles of 128
                for a in range(4):              # quad-2 normalize, chunk jj
                    pb = 64 * (a % 2)
                    dst = OTSBq[2][pb:pb + 64, a // 2,
                                   128 * jj:128 * (jj + 1)]
                    nc.vector.tensor_mul(
                        dst, dst, rb4_g2[pb:pb + 64, a,
                                         128 * jj:128 * (jj + 1)])
                pss = []
                for nh in range(2):
                    ps = P_PD.tile([128, 384], f32, tag=f"ops{nh}")
                    # two accumulation groups on one bank: slabs 0..3
                    # don't depend on quad-2's normalize, so their group
                    # runs during the normalize chain's DMA latency
                    for s in range(NDS):
                        nc.tensor.matmul(
                            ps[:],
                            OTSBq[s // 2][:, s % 2, 128 * jj:128 * (jj + 1)],
                            wo_sb[:, s, 384 * nh:384 * (nh + 1)],
                            start=(s == 0), stop=(s == 3 or s == NDS - 1),
                            skip_group_check=(s >= 4))
                    pss.append(ps)
                ob = P_O.tile([128, D], bf, tag="ob")
                nc.scalar.copy(ob[:, 0:384], pss[0][:])
                nc.scalar.copy(ob[:, 384:768], pss[1][:])
                nc.sync.dma_start(out_d[128 * jj:128 * (jj + 1), :], ob[:])

    nc.compile()
    return nc


def _prep_inputs(x, wq, wk, wv, wo, token_positions):
    import ml_dtypes
    bf16 = ml_dtypes.bfloat16

    x2 = np.ascontiguousarray(x[0], dtype=np.float32)          # [S, D]
    xT = np.ascontiguousarray(x2.T).astype(bf16)               # [D, S]
    perm = _head_perm()
    wqT = np.ascontiguousarray(wq[perm, :].T).astype(bf16)     # [d, perm e]
    wkT = np.ascontiguousarray(wk[perm, :].T).astype(bf16)
    wvT = np.ascontiguousarray(wv.T).astype(bf16)
    woT = np.ascontiguousarray(wo.T).astype(bf16)

    pos = np.asarray(token_positions[0], dtype=np.int64)       # [S]
    kk = np.arange(HD2, dtype=np.float32)
    inv = (10000.0 ** (-2.0 * kk / HD)).astype(np.float32)
    ang = pos[:, None].astype(np.float32) * inv[None, :]       # [S, 32]
    cosf = np.cos(ang, dtype=np.float32)
    sinf = np.sin(ang, dtype=np.float32)

    rc = _repl_cols()
    xTr_flat = np.ascontiguousarray(xT[:, rc])                 # [768, 2048]
    # split rows into 6 ds-slabs of 128: [128, 6, 2048]
    xTr = np.ascontiguousarray(
        xTr_flat.reshape(NDS, 128, 2048).transpose(1, 0, 2))
    cosKr = np.ascontiguousarray(np.tile(cosf[rc].T, (4, 1))).astype(bf16)
    sinKr = np.ascontiguousarray(np.tile(sinf[rc].T, (4, 1))).astype(bf16)

    per_core = []
    for c in range(NCORES):
        xkv2 = np.ascontiguousarray(
            xT[:, QPC * c + 256:QPC * (c + 1)])                # [768, 256]
        xq = np.ascontiguousarray(xT[:, c::NCORES])            # [768, 512]
        cs = slice(QPC * c + 256, QPC * (c + 1))
        cosK2 = np.ascontiguousarray(np.tile(cosf[cs].T, (4, 1))).astype(bf16)
        sinK2 = np.ascontiguousarray(np.tile(sinf[cs].T, (4, 1))).astype(bf16)
        cosQ = np.ascontiguousarray(
            np.tile(cosf[c::NCORES].T, (4, 1))).astype(bf16)
        sinQ = np.ascontiguousarray(
            np.tile(sinf[c::NCORES].T, (4, 1))).astype(bf16)
        kl = np.arange(128)[:, None]
        jj = np.arange(16)[None, :]
        mask1 = (kl <= 8 * jj + c).astype(np.float32).astype(bf16)
        mask = np.ascontiguousarray(np.concatenate([mask1] * 4, axis=1))
        per_core.append({
            "xkv2": xkv2, "xTr": xTr, "xq": xq,
            "wqT": wqT, "wkT": wkT, "wvT": wvT, "woT": woT,
            "cosK2": cosK2, "sinK2": sinK2,
            "cosKr": cosKr, "sinKr": sinKr,
            "cosQ": cosQ, "sinQ": sinQ,
            "mask": mask,
        })
    return per_core


def kernel(x, wq, wk, wv, wo, token_positions):
    global last_exec_time_ns, last_results
    import os
    from concourse import bass_utils

    key = "v3"
    if key not in _CACHE:
        _CACHE[key] = _build_program()
    nc = _CACHE[key]

    in_maps = _prep_inputs(np.asarray(x), np.asarray(wq), np.asarray(wk),
                           np.asarray(wv), np.asarray(wo),
                           np.asarray(token_positions))

    kw = {}
    if os.environ.get("BASS_KERNEL_TRACE", "0") == "1":
        kw = dict(trace=True,
                  trace_cores=[int(t) for t in os.environ.get(
                      "BASS_KERNEL_TRACE_CORES", "0").split(",")])
    res = bass_utils.run_bass_kernel_spmd(nc, in_maps,
                                          core_ids=list(range(NCORES)), **kw)
    last_exec_time_ns = res.exec_time_ns
    last_results = res

    out = np.empty((S, D), dtype=np.float32)
    for c in range(NCORES):
        out[c::NCORES, :] = np.asarray(res.results[c]["out"]).astype(
            np.float32)
    return out[None, :, :]
